# revision 16
# baseline (speedup 1.0000x reference)
"""Trainium2 Bass kernel: dense-CRF mean-field layer (96x96, 21 labels).

fp8 redesign of the bf16 baseline (361908 ns -> ~298000 ns modeled):
  * K held in SBUF as fp8-e4m3 hi/lo pairs; the d2 feature matmul directly
    emits e4m3 exponent-scale values; exp/convert work is pipelined across
    ACT (exp), ACT/Pool (fp8 convert) and DVE (residual).
  * DoubleRow fp8 bursts: iterations 0-2 stream hi-plane chunk pairs
    (2 cols/cycle), iterations 3-4 stream (hi,lo) pairs via a broadcast
    lhsT for ~bf16 accuracy where it matters.
  * q carried as fp8 + explicit residual channels (+0.125 norm channel);
    P rows recombine post-burst.
  * Iteration-0 burst fused into the K build; y-blur emits the x-major
    tail layout directly; one AllGather pair per iteration with
    native-dtype payloads and merged multi-dim-AP reloads.
"""
import sys
sys.path.insert(0, "/opt/trn_rl_repo")
import os
import numpy as np
import ml_dtypes

H = W = 96
N = H * W                  # 9216
L = 21
LE = L + 1                 # 22 (21 labels + norm channel)
ALPHA, BETA, GAMMA = 80.0, 13.0, 3.0
W_SPATIAL, W_BILATERAL = 3.0, 10.0
NUM_ITERATIONS = 5
NCORES = 8
S = N // NCORES            # 1152 columns (own pixels) per core
YPC = H // NCORES          # 12 image rows per core
CH = N // 128              # 72 chunks of 128 rows (global j)
CW = 64                    # qsb per-chunk column stride (44 used + pad)
NCH = 2 * LE               # 44 real lhs channels: q8(21) norm r8(21) zero
GY = 4                     # y rows per group/piece
GRP = YPC // GY            # 3 groups
GS = GY * W                # 384 columns per group
ONESV = 0.125              # exactly representable norm channel value
QSC = 1.25                 # q scale; ONESV * (1/QSC) * ... -> 10*avg? see below
SCB = 8.0 / np.log(2.0)    # e4m3 bits-per-octave scale
C2 = 56.5
LN2_8 = float(np.log(2.0) / 8.0)
ABIAS = float(-C2 * LN2_8 + 4.0 * np.log(2.0))   # exp -> A = 16*K (bf16)

PQ_F32 = 128 * 9 * NCH // 4       # 12672 f32 slots: fp8 q part [128, 9*44]
PT_F32 = YPC * L * W * 2 // 4     # 12096 f32 slots: bf16 t1 part [12, 21*96]
PAYP = PQ_F32 + PT_F32            # 24768 f32 payload per core

LAST_EXEC_NS = None
_CACHE = {}
import os as _os
DBG_SIM = bool(int(_os.environ.get("CRF_DBG_SIM", "0")))


def _iter_pairs(double):
    """Burst pair list [(vA, vB, qA, qB)] in reload (core) arrival order.
    v* are K2 virtual-chunk indices (2c=hi, 2c+1=lo), q* are qsb chunks."""
    pairs = []
    for r in range(NCORES):
        if double:
            for j in range(9):
                c = 9 * r + j
                pairs.append((2 * c, 2 * c + 1, c, c))
        else:
            for j in range(4):
                c = 9 * r + 2 * j
                pairs.append((2 * c, 2 * (c + 1), c, c + 1))
    if not double:
        lefts = [9 * r + 8 for r in range(NCORES)]
        for k in range(0, NCORES, 2):
            pairs.append((2 * lefts[k], 2 * lefts[k + 1],
                          lefts[k], lefts[k + 1]))
    return pairs


# partition-shift pieces for the q shuffle of one 4-row group:
# (y_off, x0, n, chunk_off, p0): rows 96*y_off+x -> piece chunk c, partition p
SHUF = ((0, 0, 96, 0, 0), (1, 0, 32, 0, 96), (1, 32, 64, 1, 0),
        (2, 0, 64, 1, 64), (2, 64, 32, 2, 0), (3, 0, 96, 2, 32))


def _build_bass(sim1=False):
    key = "nc_sim1" if sim1 else "nc"
    if key in _CACHE:
        return _CACHE[key]
    import concourse.bass as bass  # noqa: F401
    from concourse import bacc
    import concourse.mybir as mybir
    import concourse.tile as tile

    f32 = mybir.dt.float32
    bf16 = mybir.dt.bfloat16
    fp8 = mybir.dt.float8e4
    AF = mybir.ActivationFunctionType
    OP = mybir.AluOpType
    AX = mybir.AxisListType
    DR = mybir.MatmulPerfMode.DoubleRow

    nc = bacc.Bacc("TRN2", target_bir_lowering=False, debug=False,
                   num_devices=1 if sim1 else NCORES)

    featL_d = nc.dram_tensor("featL", [21, N], bf16, kind="ExternalInput")
    featR_d = nc.dram_tensor("featR", [21, S], bf16, kind="ExternalInput")
    uTx_d = nc.dram_tensor("uTx", [W, YPC * LE], f32, kind="ExternalInput")
    Ax_d = nc.dram_tensor("Ax", [W, W], bf16, kind="ExternalInput")
    Ay_d = nc.dram_tensor("Ay", [H, YPC], bf16, kind="ExternalInput")
    qsb0_d = nc.dram_tensor("qsb0", [128, CH * CW], fp8, kind="ExternalInput")
    t1f0_d = nc.dram_tensor("t1f0", [H, L * W], bf16, kind="ExternalInput")
    id_d = nc.dram_tensor("ident", [LE, LE], f32, kind="ExternalInput")
    qout_d = nc.dram_tensor("qout", [S, L], f32, kind="ExternalOutput")
    dbg = bool(int(os.environ.get("CRF_DEBUG", "0")))
    if dbg:
        dbg_k2 = nc.dram_tensor("dbg_k2", [128, 2 * CH * S], fp8,
                                kind="ExternalOutput")
        dbg_pev = nc.dram_tensor("dbg_pev", [2 * LE, GS], f32,
                                 kind="ExternalOutput")
        dbg_qy = nc.dram_tensor("dbg_qy", [W, YPC * LE], f32,
                                kind="ExternalOutput")
        dbg_t2s = nc.dram_tensor("dbg_t2s", [W, L * YPC], f32,
                                 kind="ExternalOutput")
        dbg_qsb = nc.dram_tensor("dbg_qsb", [128, CH * CW], fp8,
                                 kind="ExternalOutput")
        dbg_t1n = nc.dram_tensor("dbg_t1n", [H, L * W], bf16,
                                 kind="ExternalOutput")
        dbg_t1s = nc.dram_tensor("dbg_t1s", [L, S], bf16,
                                 kind="ExternalOutput")
        dbg_qyb = nc.dram_tensor("dbg_qyb", [W, YPC * L], bf16,
                                 kind="ExternalOutput")

    with tile.TileContext(nc) as tc:
        with (
            tc.tile_pool(name="const", bufs=1) as constp,
            tc.tile_pool(name="kbl", bufs=1) as kblp,
            tc.tile_pool(name="work", bufs=1) as work,
            tc.tile_pool(name="dram", bufs=2, space="DRAM") as dram,
        ):
            Ax = constp.tile([W, W], bf16)
            nc.sync.dma_start(Ax[:], Ax_d[:])
            Ay = constp.tile([H, YPC], bf16)
            nc.sync.dma_start(Ay[:], Ay_d[:])
            uTx = constp.tile([W, YPC * LE], f32)
            nc.sync.dma_start(uTx[:], uTx_d[:])
            idn = constp.tile([LE, LE], f32)
            nc.sync.dma_start(idn[:], id_d[:])
            featR = constp.tile([21, S], bf16)
            nc.sync.dma_start(featR[:], featR_d[:])
            abias = constp.tile([128, 1], f32)
            nc.vector.memset(abias[:], ABIAS)

            K2 = kblp.tile([128, 2 * CH * S], fp8)      # 162 KB hi/lo pairs
            K2r = K2[:].rearrange("p (v s) -> p v s", s=S)

            qsb0 = work.tile([128, CH * CW], fp8, tag="qsb", bufs=2)
            nc.sync.dma_start(qsb0[:], qsb0_d[:])
            t1f0 = work.tile([H, L * W], bf16, tag="t1full", bufs=2)
            nc.sync.dma_start(t1f0[:], t1f0_d[:])

            # P psum per output group, opened before the build pools so the
            # fused iteration-0 burst can accumulate through the build.
            psP_ctx = tc.tile_pool(name="psP", bufs=1, space="PSUM")
            psP = psP_ctx.__enter__()
            P0 = [psP.tile([CW, GS], f32, tag=f"P{g}", name=f"P{g}")
                  for g in range(GRP)]

            # ---------- K build: y = c2 - (SCB/2) d2 ; hi/lo e4m3 ----------
            q0V = qsb0[:].rearrange("p (c l) -> p c l", l=CW)
            with (
                tc.tile_pool(name="pre_sb", bufs=2) as pre_sb,
                tc.tile_pool(name="pre_ps", bufs=2, space="PSUM") as pre_ps,
            ):
                for cp in range(CH // 2):
                    flb = pre_sb.tile([21, 256], bf16, tag="fl")
                    nc.sync.dma_start(flb[:],
                                      featL_d[:, 256 * cp:256 * (cp + 1)])
                    A = pre_sb.tile([128, 2304], bf16, tag="A")
                    for wi in range(3):
                        off = wi * 768
                        d2 = pre_ps.tile([128, 768], f32, tag="d2")
                        # cut at the psum bank boundary (window-relative
                        # col 512) and at the chunk boundary (global 1152)
                        cuts = sorted({off, off + 768, off + 512}
                                      | ({1152} if off < 1152 < off + 768
                                         else set()))
                        for a, b in zip(cuts[:-1], cuts[1:]):
                            cj = a // 1152          # 0/1: chunk within pair
                            il = a - cj * 1152      # local i col
                            nc.tensor.matmul(
                                d2[:, a - off:b - off],
                                flb[:, cj * 128:(cj + 1) * 128],
                                featR[:, il:il + (b - a)],
                                start=True, stop=True)
                        nc.scalar.activation(A[:, off:off + 768],
                                             d2[:, 0:768], AF.Exp,
                                             bias=abias[:], scale=LN2_8)
                    for j in (0, 1):
                        c = 2 * cp + j
                        hi = K2r[:, 2 * c, :]
                        lo = K2r[:, 2 * c + 1, :]
                        Aj = A[:, j * 1152:(j + 1) * 1152]
                        if c % 4 == 3:
                            nc.scalar.copy(hi, Aj)
                        else:
                            nc.gpsimd.tensor_copy(hi, Aj)
                        nc.vector.tensor_tensor(lo, Aj, hi, OP.subtract)
                    # fused iteration-0 burst, lagging 2 super-steps so PE
                    # never waits on the fp8 converts
                    nofuse = bool(int(os.environ.get("CRF_NOFUSE", "0")))
                    if not nofuse and cp >= 2:
                        bp = cp - 2
                        for g in range(GRP):
                            nc.tensor.matmul(
                                P0[g][:, 0:GS],
                                q0V[:, 2 * bp:2 * bp + 2, :],
                                K2r[:, 4 * bp:4 * bp + 3:2,
                                    g * GS:(g + 1) * GS],
                                start=(bp == 0), stop=False,
                                perf_mode=DR)
                tail_bps = (list(range(CH // 2)) if nofuse
                            else [CH // 2 - 2, CH // 2 - 1])
                for bp in tail_bps:
                    for g in range(GRP):
                        nc.tensor.matmul(
                            P0[g][:, 0:GS],
                            q0V[:, 2 * bp:2 * bp + 2, :],
                            K2r[:, 4 * bp:4 * bp + 3:2,
                                g * GS:(g + 1) * GS],
                            start=(nofuse and bp == 0) or
                                  (not nofuse and False),
                            stop=(bp == CH // 2 - 1),
                            perf_mode=DR)

            if dbg:
                nc.sync.dma_start(dbg_k2.ap(), K2[:])

            # ---------- mean-field iterations ----------
            with (
                tc.tile_pool(name="ps_t2", bufs=1, space="PSUM") as ps_t2,
                tc.tile_pool(name="ps_tp", bufs=2, space="PSUM") as ps_tp,
                tc.tile_pool(name="ps_xb", bufs=2, space="PSUM") as ps_xb,
            ):
                qsb_cur = qsb0
                t1_cur = t1f0
                for it in range(NUM_ITERATIONS):
                    last = it == NUM_ITERATIONS - 1
                    double = it >= 3
                    # burst (iteration 0 already accumulated during build)
                    if it > 0:
                        qV = qsb_cur[:].rearrange("p (c l) -> p c l", l=CW)
                        pairs = _iter_pairs(double)
                        for g in range(GRP):
                            for pi, (vA, vB, qA, qB) in enumerate(pairs):
                                rhs = K2r[:, vA:vB + 1:(vB - vA),
                                          g * GS:(g + 1) * GS]
                                if qA == qB:
                                    lhs = (qV[:, qA:qA + 1, :]
                                           .to_broadcast([128, 2, CW]))
                                else:
                                    lhs = qV[:, qA:qB + 1:(qB - qA), :]
                                nc.tensor.matmul(
                                    P0[g][:, 0:GS], lhs, rhs,
                                    start=(pi == 0),
                                    stop=(pi == len(pairs) - 1),
                                    perf_mode=DR)

                    # y-blur direct to x-major: t2x[x, 12l+y'] per label
                    t2x = ps_t2.tile([W, L * YPC], f32, tag="t2x")
                    for l in range(L):
                        nc.tensor.matmul(t2x[:, l * YPC:(l + 1) * YPC],
                                         t1_cur[:, l * W:(l + 1) * W],
                                         Ay[:], start=True, stop=True)
                    t2s = work.tile([W, L * YPC], f32, tag="t2s")
                    nc.scalar.copy(t2s[:], t2x[:])

                    if not last:
                        qsb_nxt = work.tile([128, CH * CW], fp8, tag="qsb", bufs=2)
                        t1_nxt = work.tile([H, L * W], bf16, tag="t1full", bufs=2)
                        if it < 2:
                            # zero the pad columns once per buffer (lhsT
                            # reads 64-wide; NaN bit patterns must not leak)
                            qpad = (qsb_nxt[:]
                                    .rearrange("p (c b l) -> p c b l", b=2,
                                               l=32))
                            nc.vector.memset(qpad[:, :, 0, LE:32], 0.0)
                            nc.vector.memset(qpad[:, :, 1, LE:32], 0.0)

                    qy = work.tile([W, YPC * LE], f32, tag="qy")
                    qyv = qy[:].rearrange("x (y l) -> x y l", l=LE)
                    rec = work.tile([W, YPC], f32, tag="rec")
                    ssum = work.tile([W, YPC], f32, tag="ssum")
                    if not last:
                        qyb = work.tile([W, YPC * L], bf16, tag="qyb")
                        q8r8 = work.tile([W, YPC * NCH], fp8, tag="q8r8")
                        q8a = q8r8[:].rearrange("x (y h l) -> x y h l", h=2,
                                                l=LE)
                        q8v = q8a[:, :, 0, :]
                        r8v = q8a[:, :, 1, :]
                        t1s = work.tile([L, N // NCORES], bf16, tag="t1s")
                        if it == 0:
                            nc.vector.memset(q8v[:, :, L:LE], ONESV)
                            nc.vector.memset(r8v[:, :, L:LE], 0.0)

                    # fully batched tail: recombine all groups, 12
                    # transposes, then one wide op per stage
                    tp = ps_tp.tile([W, YPC * LE], f32, tag="tp")
                    tpv = tp[:].rearrange("x (y l) -> x y l", l=LE)
                    for g in range(GRP):
                        pevA = work.tile([LE, GS], f32, tag="pevA", bufs=2,
                                         name="pevA")
                        nc.scalar.copy(pevA[:], P0[g][0:LE, 0:GS])
                        pevB = work.tile([LE, GS], f32, tag="pevB", bufs=2,
                                         name="pevB")
                        nc.scalar.copy(pevB[:], P0[g][32:32 + LE, 0:GS])
                        pbs = work.tile([LE, GS], f32, tag="pbs", bufs=2,
                                        name="pbs")
                        nc.vector.tensor_tensor(pbs[:], pevA[:], pevB[:],
                                                OP.add)
                        if dbg and it == 0 and g == 0:
                            nc.sync.dma_start(dbg_pev.ap()[0:LE, :], pevA[:])
                            nc.sync.dma_start(dbg_pev.ap()[LE:2 * LE, :],
                                              pevB[:])
                        for k in range(GY):
                            y = g * GY + k
                            nc.tensor.transpose(tp[:, y * LE:(y + 1) * LE],
                                                pbs[:, k * W:(k + 1) * W],
                                                idn[:])
                    nc.vector.reciprocal(rec[:, :, None], tpv[:, :, L:LE])
                    nc.vector.tensor_tensor(
                        tpv[:], tpv[:],
                        rec[:, :, None].to_broadcast([W, YPC, LE]), OP.mult)
                    nc.vector.tensor_tensor(
                        tpv[:], tpv[:],
                        uTx[:].rearrange("x (y l) -> x y l", l=LE), OP.add)
                    nc.vector.tensor_tensor(
                        tpv[:, :, 0:L], tpv[:, :, 0:L],
                        t2s[:].rearrange("x (l y) -> x y l", y=YPC), OP.add)
                    nc.scalar.activation(qy[:], tp[:], AF.Exp)
                    nc.vector.reduce_sum(ssum[:], qyv[:, :, 0:L], axis=AX.X)
                    nc.vector.reciprocal(ssum[:], ssum[:])
                    nc.vector.tensor_tensor(
                        qyv[:, :, 0:L], qyv[:, :, 0:L],
                        ssum[:, :, None].to_broadcast([W, YPC, L]), OP.mult)
                    if dbg and it == 0:
                        nc.sync.dma_start(dbg_qy.ap(), qy[:])
                        nc.sync.dma_start(dbg_t2s.ap(), t2s[:])
                    if dbg and it == 0 and not last:
                        nc.vector.engine_nop()

                    if not last:
                        # conversions for x-blur + fp8 payload
                        nc.vector.tensor_copy(
                            qyb[:].rearrange("x (y l) -> x y l", l=L),
                            qyv[:, :, 0:L])
                        nc.vector.tensor_scalar(
                            q8v[:, :, 0:L], qyv[:, :, 0:L], QSC, None,
                            OP.mult)
                        nc.vector.scalar_tensor_tensor(
                            r8v[:, :, 0:L], qyv[:, :, 0:L], QSC,
                            q8v[:, :, 0:L], OP.mult, OP.subtract)
                        # x-blur -> t1 staging (bf16), one psum+copy per group
                        for g in range(GRP):
                            xb = ps_xb.tile([L, GS], f32, tag="xb")
                            for k in range(GY):
                                y = g * GY + k
                                nc.tensor.matmul(
                                    xb[:, k * W:(k + 1) * W],
                                    qyb[:, y * L:(y + 1) * L], Ax[:],
                                    start=True, stop=True)
                            nc.scalar.copy(
                                t1s[:, g * GS:(g + 1) * GS], xb[:])

                    if last:
                        nc.sync.dma_start(
                            qout_d.ap()
                            .rearrange("(y x) l -> x y l", x=W),
                            qyv[:, :, 0:L])
                        continue

                    # payload: q shuffle straight to DRAM (6 partition-shift
                    # pieces, all 3 groups folded) + one t1 DMA.  q and t1
                    # travel as separate NATIVE-dtype tensors: packing bf16
                    # pairs into f32 via bitcast corrupts values whose f32
                    # interpretation is special (seen as scattered inf/NaN in
                    # the low half of f32 lanes after the collective).
                    plq = dram.tile([128, 9 * NCH], fp8, tag="plq")
                    plqv = plq[:].rearrange("p (c l) -> p c l", l=NCH)
                    for si, (yo, x0, n, co, p0) in enumerate(SHUF):
                        eng = nc.scalar if si % 2 else nc.sync
                        eng.dma_start(
                            plqv[p0:p0 + n, co::3, :],
                            q8r8[x0:x0 + n, :]
                            .rearrange("x (y hl) -> x y hl", hl=NCH)
                            [:, yo::GY, :])
                    plt = dram.tile([YPC, L * W], bf16, tag="plt")
                    # read t1s with its PLAIN 2-d AP (rearranged source reads
                    # miss subtile deps against the per-group x-blur copies);
                    # the y-major transpose lives on the DRAM side instead
                    nc.scalar.dma_start(
                        plt[:].rearrange("y (l x) -> l y x", l=L),
                        t1s[:])

                    qagq = dram.tile([NCORES * 128, 9 * NCH], fp8, tag="qagq")
                    qagt = dram.tile([NCORES * YPC, L * W], bf16, tag="qagt")
                    if sim1:
                        for r in range(NCORES):
                            nc.sync.dma_start(
                                qagq[128 * r:128 * (r + 1), :], plq[:])
                            nc.sync.dma_start(
                                qagt[YPC * r:YPC * (r + 1), :], plt[:])
                    else:
                        nc.gpsimd.collective_compute(
                            "AllGather", OP.bypass,
                            replica_groups=[list(range(NCORES))],
                            ins=[plq.opt()], outs=[qagq.opt()])
                        nc.gpsimd.collective_compute(
                            "AllGather", OP.bypass,
                            replica_groups=[list(range(NCORES))],
                            ins=[plt.opt()], outs=[qagt.opt()])

                    # reloads for the next iteration, one q DMA per core (so
                    # the next burst starts as soon as core 0's slab lands)
                    for r in range(NCORES):
                        eng = nc.sync if r % 2 else nc.scalar
                        eng.dma_start(
                            qsb_nxt[:, 9 * r * CW:(9 * r + 9) * CW]
                            .rearrange("p (c b l) -> p c b l", b=2, l=32)
                            [:, :, :, 0:LE],
                            qagq[128 * r:128 * (r + 1), :]
                            .rearrange("p (c h l) -> p c h l", h=2, l=LE))
                    nc.sync.dma_start(t1_nxt[:], qagt[:])
                    if dbg and it == 0:
                        nc.vector.engine_nop()
                        nc.sync.dma_start(dbg_qsb.ap(), qsb_nxt[:])
                        nc.sync.dma_start(dbg_t1n.ap(), t1_nxt[:])
                        nc.sync.dma_start(dbg_t1s.ap(), t1s[:])
                        nc.sync.dma_start(dbg_qyb.ap(), qyb[:])
                    qsb_cur = qsb_nxt
                    t1_cur = t1_nxt
            psP_ctx.__exit__(None, None, None)

    nc.compile()
    _CACHE[key] = nc
    return nc


def _host_prepare(unaries, rgb):
    e4m3 = ml_dtypes.float8_e4m3
    bfd = ml_dtypes.bfloat16
    u = np.asarray(unaries, np.float32).reshape(N, L)
    c = np.asarray(rgb, np.float32).reshape(N, 3)

    ys, xs = np.meshgrid(np.arange(H, dtype=np.float64),
                         np.arange(W, dtype=np.float64), indexing="ij")
    pos = np.stack([ys.ravel(), xs.ravel()], -1)
    g = np.concatenate([c.astype(np.float64) / BETA, pos / ALPHA], 1)
    g = g - g.mean(0, keepdims=True)
    sq = (g * g).sum(1)
    ones = np.ones(N, np.float64)
    # y_ji = (SCB*g_j).g_i - SCB/2*sq_j + (-SCB/2*sq_i + C2)
    L7 = np.concatenate([(SCB * g).T, (-(SCB / 2) * sq)[None], ones[None]], 0)
    R7 = np.concatenate([g.T, ones[None], ((-(SCB / 2) * sq) + C2)[None]], 0)

    def split(A7):
        hi = A7.astype(bfd)
        lo = (A7 - hi.astype(np.float64)).astype(bfd)
        return hi, lo

    Lhi, Llo = split(L7)
    Rhi, Rlo = split(R7)
    featL = np.ascontiguousarray(np.concatenate([Lhi, Lhi, Llo], 0))  # [21,N]
    featR = np.ascontiguousarray(np.concatenate([Rhi, Rlo, Rhi], 0))

    d = np.arange(W, dtype=np.float64)
    A = np.exp(-(d[:, None] - d[None, :]) ** 2 / (2.0 * GAMMA * GAMMA))
    nvec = A.sum(0)
    Ax = np.ascontiguousarray((A / nvec[None, :]).astype(bfd))

    um = u.max(1, keepdims=True)
    e = np.exp(u - um)
    q0 = e / e.sum(1, keepdims=True)

    q8 = (QSC * q0).astype(e4m3)
    r8 = (QSC * q0 - q8.astype(np.float32)).astype(e4m3)
    q44 = np.zeros((N, CW), e4m3)
    q44[:, 0:L] = q8
    q44[:, L] = np.float32(ONESV)
    q44[:, 32:32 + L] = r8
    qsb0 = np.ascontiguousarray(
        q44.reshape(CH, 128, CW).transpose(1, 0, 2).reshape(128, CH * CW))

    q3 = q0.astype(bfd).astype(np.float64).reshape(H, W, L)
    t1 = np.einsum("Xx,yXl->ylx", A / nvec[None, :], q3)      # [96, 21, 96]
    t1f0 = np.ascontiguousarray(t1.reshape(H, L * W).astype(bfd))

    ident = np.eye(LE, dtype=np.float32)

    in_maps = []
    for core in range(NCORES):
        rows = slice(core * S, (core + 1) * S)
        yc = slice(core * YPC, (core + 1) * YPC)
        Ay_c = np.ascontiguousarray(
            (A[:, yc] * (W_SPATIAL / nvec[yc])[None, :]).astype(bfd))
        # x-major unaries: uTx[x, y_loc*22 + l], channel 21 = -50
        uc = u[rows].reshape(YPC, W, L)
        uTx = np.full((W, YPC, LE), -50.0, np.float32)
        uTx[:, :, 0:L] = uc.transpose(1, 0, 2)
        in_maps.append({
            "featL": featL,
            "featR": np.ascontiguousarray(featR[:, rows]),
            "uTx": np.ascontiguousarray(uTx.reshape(W, YPC * LE)),
            "Ax": Ax,
            "Ay": Ay_c,
            "qsb0": qsb0,
            "t1f0": t1f0,
            "ident": ident,
        })
    return in_maps


def _get_runner():
    if "runner" in _CACHE:
        return _CACHE["runner"]
    import jax
    from jax.sharding import Mesh, PartitionSpec
    from jax.experimental.shard_map import shard_map
    import concourse.mybir as mybir
    from concourse import bass2jax

    nc = _build_bass()
    if not DBG_SIM:
        bass2jax.install_neuronx_cc_hook()

    partition_name = (nc.partition_id_tensor.name
                      if nc.partition_id_tensor else None)
    in_names, out_names, out_avals, zero_outs = [], [], [], []
    for alloc in nc.m.functions[0].allocations:
        if not isinstance(alloc, mybir.MemoryLocationSet):
            continue
        name = alloc.memorylocations[0].name
        if alloc.kind == "ExternalInput":
            if name != partition_name:
                in_names.append(name)
        elif alloc.kind == "ExternalOutput":
            shape = tuple(alloc.tensor_shape)
            dtype = mybir.dt.np(alloc.dtype)
            out_names.append(name)
            out_avals.append(jax.core.ShapedArray(shape, dtype))
            zero_outs.append(np.zeros(shape, dtype))
    n_params = len(in_names)
    all_in_names = list(in_names) + list(out_names)
    if partition_name is not None:
        all_in_names.append(partition_name)

    def _body(*args):
        operands = list(args)
        if partition_name is not None:
            operands.append(bass2jax.partition_id_tensor())
        outs = bass2jax._bass_exec_p.bind(
            *operands,
            out_avals=tuple(out_avals),
            in_names=tuple(all_in_names),
            out_names=tuple(out_names),
            lowering_input_output_aliases=(),
            sim_require_finite=DBG_SIM,
            sim_require_nnan=False,
            nc=nc,
        )
        return tuple(outs)

    devices = jax.devices()[:NCORES]
    mesh = Mesh(np.asarray(devices), ("core",))
    n_outs = len(out_names)
    in_specs = (PartitionSpec("core"),) * (n_params + n_outs)
    out_specs = (PartitionSpec("core"),) * n_outs
    donate = tuple(range(n_params, n_params + n_outs))
    fn = jax.jit(
        shard_map(_body, mesh=mesh, in_specs=in_specs, out_specs=out_specs,
                  check_rep=False),
        donate_argnums=donate, keep_unused=True)
    _CACHE["runner"] = (fn, in_names, out_names, out_avals, zero_outs)
    return _CACHE["runner"]


def _concat_inputs(in_maps, in_names):
    return [np.concatenate([np.asarray(in_maps[c][nm]) for c in range(NCORES)],
                           axis=0) for nm in in_names]


def _run(in_maps):
    fn, in_names, out_names, out_avals, zero_outs = _get_runner()
    concat_in = _concat_inputs(in_maps, in_names)
    concat_zeros = [np.zeros((NCORES * z.shape[0], *z.shape[1:]), z.dtype)
                    for z in zero_outs]
    out_arrs = fn(*concat_in, *concat_zeros)
    return out_arrs, out_names, out_avals


def kernel(unaries, rgb):
    in_maps = _host_prepare(unaries, rgb)
    out_arrs, out_names, out_avals = _run(in_maps)
    qi = out_names.index("qout")
    q = np.asarray(out_arrs[qi]).reshape(NCORES, S, L).reshape(N, L)
    return np.ascontiguousarray(q[None].astype(np.float32))


def time_kernel(unaries, rgb, iters=20):
    """Steady-state per-call wall time of the compiled 8-core executable."""
    import time as _time
    import jax
    in_maps = _host_prepare(unaries, rgb)
    fn, in_names, out_names, out_avals, zero_outs = _get_runner()
    concat_in = _concat_inputs(in_maps, in_names)

    def once():
        concat_zeros = [np.zeros((NCORES * z.shape[0], *z.shape[1:]), z.dtype)
                        for z in zero_outs]
        outs = fn(*concat_in, *concat_zeros)
        jax.block_until_ready(outs)
        return outs

    once()
    times = []
    for _ in range(iters):
        t0 = _time.perf_counter()
        once()
        times.append(_time.perf_counter() - t0)
    return min(times), sorted(times)[len(times) // 2]


# revision 26
# speedup vs baseline: 1.0456x; 1.0456x over previous
"""Trainium2 Bass kernel: dense-CRF mean-field layer (96x96, 21 labels).

fp8 redesign of the bf16 baseline (361908 ns -> ~298000 ns modeled):
  * K held in SBUF as fp8-e4m3 hi/lo pairs; the d2 feature matmul directly
    emits e4m3 exponent-scale values; exp/convert work is pipelined across
    ACT (exp), ACT/Pool (fp8 convert) and DVE (residual).
  * DoubleRow fp8 bursts: iterations 0-2 stream hi-plane chunk pairs
    (2 cols/cycle), iterations 3-4 stream (hi,lo) pairs via a broadcast
    lhsT for ~bf16 accuracy where it matters.
  * q carried as fp8 + explicit residual channels (+0.125 norm channel);
    P rows recombine post-burst.
  * Iteration-0 burst fused into the K build; y-blur emits the x-major
    tail layout directly; one AllGather pair per iteration with
    native-dtype payloads and merged multi-dim-AP reloads.
"""
import sys
sys.path.insert(0, "/opt/trn_rl_repo")
import os
import numpy as np
import ml_dtypes

H = W = 96
N = H * W                  # 9216
L = 21
LE = L + 1                 # 22 (21 labels + norm channel)
ALPHA, BETA, GAMMA = 80.0, 13.0, 3.0
W_SPATIAL, W_BILATERAL = 3.0, 10.0
NUM_ITERATIONS = 5
NCORES = 8
S = N // NCORES            # 1152 columns (own pixels) per core
YPC = H // NCORES          # 12 image rows per core
CH = N // 128              # 72 chunks of 128 rows (global j)
CW = 64                    # qsb per-chunk column stride (44 used + pad)
NCH = 2 * LE               # 44 real lhs channels: q8(21) norm r8(21) zero
GY = 4                     # y rows per group/piece
GRP = YPC // GY            # 3 groups
GS = GY * W                # 384 columns per group
ONESV = 0.125              # exactly representable norm channel value
QSC = 1.25                 # q scale; ONESV * (1/QSC) * ... -> 10*avg? see below
SCB = 8.0 / np.log(2.0)    # e4m3 bits-per-octave scale
C2 = 56.5
LN2_8 = float(np.log(2.0) / 8.0)
ABIAS = float(-C2 * LN2_8 + 4.0 * np.log(2.0))   # exp -> A = 16*K (bf16)

PQ_F32 = 128 * 9 * NCH // 4       # 12672 f32 slots: fp8 q part [128, 9*44]
PT_F32 = YPC * L * W * 2 // 4     # 12096 f32 slots: bf16 t1 part [12, 21*96]
PAYP = PQ_F32 + PT_F32            # 24768 f32 payload per core

LAST_EXEC_NS = None
_CACHE = {}
import os as _os
DBG_SIM = bool(int(_os.environ.get("CRF_DBG_SIM", "0")))


def _iter_pairs(double):
    """Burst pair list [(vA, vB, qA, qB)] in reload (core) arrival order.
    v* are K2 virtual-chunk indices (2c=hi, 2c+1=lo), q* are qsb chunks."""
    pairs = []
    for r in range(NCORES):
        if double:
            for j in range(9):
                c = 9 * r + j
                pairs.append((2 * c, 2 * c + 1, c, c))
        else:
            for j in range(4):
                c = 9 * r + 2 * j
                pairs.append((2 * c, 2 * (c + 1), c, c + 1))
    if not double:
        lefts = [9 * r + 8 for r in range(NCORES)]
        for k in range(0, NCORES, 2):
            pairs.append((2 * lefts[k], 2 * lefts[k + 1],
                          lefts[k], lefts[k + 1]))
    return pairs


# partition-shift pieces for the q shuffle of one 4-row group:
# (y_off, x0, n, chunk_off, p0): rows 96*y_off+x -> piece chunk c, partition p
SHUF = ((0, 0, 96, 0, 0), (1, 0, 32, 0, 96), (1, 32, 64, 1, 0),
        (2, 0, 64, 1, 64), (2, 64, 32, 2, 0), (3, 0, 96, 2, 32))


def _build_bass(sim1=False):
    key = "nc_sim1" if sim1 else "nc"
    if key in _CACHE:
        return _CACHE[key]
    import concourse.bass as bass  # noqa: F401
    from concourse import bacc
    import concourse.mybir as mybir
    import concourse.tile as tile

    f32 = mybir.dt.float32
    bf16 = mybir.dt.bfloat16
    fp8 = mybir.dt.float8e4
    AF = mybir.ActivationFunctionType
    OP = mybir.AluOpType
    AX = mybir.AxisListType
    DR = mybir.MatmulPerfMode.DoubleRow

    nc = bacc.Bacc("TRN2", target_bir_lowering=False, debug=False,
                   num_devices=1 if sim1 else NCORES)

    featL_d = nc.dram_tensor("featL", [21, N], bf16, kind="ExternalInput")
    featR_d = nc.dram_tensor("featR", [21, S], bf16, kind="ExternalInput")
    uTx_d = nc.dram_tensor("uTx", [W, YPC * LE], f32, kind="ExternalInput")
    Ax_d = nc.dram_tensor("Ax", [W, W], bf16, kind="ExternalInput")
    Ay_d = nc.dram_tensor("Ay", [H, YPC], bf16, kind="ExternalInput")
    qsb0_d = nc.dram_tensor("qsb0", [128, CH * CW], fp8, kind="ExternalInput")
    t1f0_d = nc.dram_tensor("t1f0", [H, L * W], bf16, kind="ExternalInput")
    id_d = nc.dram_tensor("ident", [LE, LE], f32, kind="ExternalInput")
    qout_d = nc.dram_tensor("qout", [S, L], f32, kind="ExternalOutput")
    dbg = bool(int(os.environ.get("CRF_DEBUG", "0")))
    if dbg:
        dbg_k2 = nc.dram_tensor("dbg_k2", [128, 2 * CH * S], fp8,
                                kind="ExternalOutput")
        dbg_pev = nc.dram_tensor("dbg_pev", [2 * LE, GS], f32,
                                 kind="ExternalOutput")
        dbg_qy = nc.dram_tensor("dbg_qy", [W, YPC * LE], f32,
                                kind="ExternalOutput")
        dbg_t2s = nc.dram_tensor("dbg_t2s", [W, L * YPC], f32,
                                 kind="ExternalOutput")
        dbg_qsb = nc.dram_tensor("dbg_qsb", [128, CH * CW], fp8,
                                 kind="ExternalOutput")
        dbg_t1n = nc.dram_tensor("dbg_t1n", [H, L * W], bf16,
                                 kind="ExternalOutput")
        dbg_t1s = nc.dram_tensor("dbg_t1s", [L, S], bf16,
                                 kind="ExternalOutput")
        dbg_qyb = nc.dram_tensor("dbg_qyb", [W, YPC * L], bf16,
                                 kind="ExternalOutput")

    with tile.TileContext(nc) as tc:
        with (
            tc.tile_pool(name="const", bufs=1) as constp,
            tc.tile_pool(name="kbl", bufs=1) as kblp,
            tc.tile_pool(name="work", bufs=1) as work,
            tc.tile_pool(name="dram", bufs=2, space="DRAM") as dram,
        ):
            Ax = constp.tile([W, W], bf16)
            nc.sync.dma_start(Ax[:], Ax_d[:])
            Ay = constp.tile([H, YPC], bf16)
            nc.sync.dma_start(Ay[:], Ay_d[:])
            uTx = constp.tile([W, YPC * LE], f32)
            nc.sync.dma_start(uTx[:], uTx_d[:])
            idn = constp.tile([LE, LE], f32)
            nc.sync.dma_start(idn[:], id_d[:])
            featR = constp.tile([21, S], bf16)
            nc.sync.dma_start(featR[:], featR_d[:])
            abias = constp.tile([128, 1], f32)
            nc.vector.memset(abias[:], ABIAS)

            K2 = kblp.tile([128, 2 * CH * S], fp8)      # 162 KB hi/lo pairs
            K2r = K2[:].rearrange("p (v s) -> p v s", s=S)

            qsb0 = work.tile([128, CH * CW], fp8, tag="qsb", bufs=2)
            nc.sync.dma_start(qsb0[:], qsb0_d[:])
            t1f0 = work.tile([H, L * W], bf16, tag="t1full", bufs=2)
            nc.sync.dma_start(t1f0[:], t1f0_d[:])

            # P psum per output group, opened before the build pools so the
            # fused iteration-0 burst can accumulate through the build.
            psP_ctx = tc.tile_pool(name="psP", bufs=1, space="PSUM")
            psP = psP_ctx.__enter__()
            P0 = [psP.tile([CW, GS], f32, tag=f"P{g}", name=f"P{g}")
                  for g in range(GRP)]

            # ---------- K build: y = c2 - (SCB/2) d2 ; hi/lo e4m3 ----------
            q0V = qsb0[:].rearrange("p (c l) -> p c l", l=CW)
            with (
                tc.tile_pool(name="pre_sb", bufs=2) as pre_sb,
                tc.tile_pool(name="pre_ps", bufs=2, space="PSUM") as pre_ps,
            ):
                for cp in range(CH // 2):
                    flb = pre_sb.tile([21, 256], bf16, tag="fl")
                    nc.sync.dma_start(flb[:],
                                      featL_d[:, 256 * cp:256 * (cp + 1)])
                    A = pre_sb.tile([128, 2304], bf16, tag="A")
                    for wi in range(3):
                        off = wi * 768
                        d2 = pre_ps.tile([128, 768], f32, tag="d2")
                        # cut at the psum bank boundary (window-relative
                        # col 512) and at the chunk boundary (global 1152)
                        cuts = sorted({off, off + 768, off + 512}
                                      | ({1152} if off < 1152 < off + 768
                                         else set()))
                        for a, b in zip(cuts[:-1], cuts[1:]):
                            cj = a // 1152          # 0/1: chunk within pair
                            il = a - cj * 1152      # local i col
                            nc.tensor.matmul(
                                d2[:, a - off:b - off],
                                flb[:, cj * 128:(cj + 1) * 128],
                                featR[:, il:il + (b - a)],
                                start=True, stop=True)
                        nc.scalar.activation(A[:, off:off + 768],
                                             d2[:, 0:768], AF.Exp,
                                             bias=abias[:], scale=LN2_8)
                    for j in (0, 1):
                        c = 2 * cp + j
                        hi = K2r[:, 2 * c, :]
                        lo = K2r[:, 2 * c + 1, :]
                        Aj = A[:, j * 1152:(j + 1) * 1152]
                        if c % 4 == 3:
                            nc.scalar.copy(hi, Aj)
                        else:
                            nc.gpsimd.tensor_copy(hi, Aj)
                        nc.vector.tensor_tensor(lo, Aj, hi, OP.subtract)
                    # fused iteration-0 burst, lagging 2 super-steps so PE
                    # never waits on the fp8 converts
                    nofuse = bool(int(os.environ.get("CRF_NOFUSE", "0")))
                    if not nofuse and cp >= 2:
                        bp = cp - 2
                        for g in range(GRP):
                            nc.tensor.matmul(
                                P0[g][:, 0:GS],
                                q0V[:, 2 * bp:2 * bp + 2, :],
                                K2r[:, 4 * bp:4 * bp + 3:2,
                                    g * GS:(g + 1) * GS],
                                start=(bp == 0), stop=False,
                                perf_mode=DR)
                tail_bps = (list(range(CH // 2)) if nofuse
                            else [CH // 2 - 2, CH // 2 - 1])
                for bp in tail_bps:
                    for g in range(GRP):
                        nc.tensor.matmul(
                            P0[g][:, 0:GS],
                            q0V[:, 2 * bp:2 * bp + 2, :],
                            K2r[:, 4 * bp:4 * bp + 3:2,
                                g * GS:(g + 1) * GS],
                            start=(nofuse and bp == 0) or
                                  (not nofuse and False),
                            stop=(bp == CH // 2 - 1),
                            perf_mode=DR)

            if dbg:
                nc.sync.dma_start(dbg_k2.ap(), K2[:])

            # ---------- mean-field iterations ----------
            with (
                tc.tile_pool(name="ps_t2", bufs=1, space="PSUM") as ps_t2,
                tc.tile_pool(name="ps_tp", bufs=2, space="PSUM") as ps_tp,
                tc.tile_pool(name="ps_xb", bufs=2, space="PSUM") as ps_xb,
            ):
                qsb_cur = qsb0
                t1_cur = t1f0
                for it in range(NUM_ITERATIONS):
                    last = it == NUM_ITERATIONS - 1
                    double = it >= 3
                    # burst (iteration 0 already accumulated during build)
                    if it > 0:
                        qV = qsb_cur[:].rearrange("p (c l) -> p c l", l=CW)
                        pairs = _iter_pairs(double)
                        for g in range(GRP):
                            for pi, (vA, vB, qA, qB) in enumerate(pairs):
                                rhs = K2r[:, vA:vB + 1:(vB - vA),
                                          g * GS:(g + 1) * GS]
                                if qA == qB:
                                    lhs = (qV[:, qA:qA + 1, :]
                                           .to_broadcast([128, 2, CW]))
                                else:
                                    lhs = qV[:, qA:qB + 1:(qB - qA), :]
                                nc.tensor.matmul(
                                    P0[g][:, 0:GS], lhs, rhs,
                                    start=(pi == 0),
                                    stop=(pi == len(pairs) - 1),
                                    perf_mode=DR)

                    # y-blur direct to x-major: t2x[x, 12l+y'] per label
                    t2x = ps_t2.tile([W, L * YPC], f32, tag="t2x")
                    for l in range(L):
                        nc.tensor.matmul(t2x[:, l * YPC:(l + 1) * YPC],
                                         t1_cur[:, l * W:(l + 1) * W],
                                         Ay[:], start=True, stop=True)
                    t2s = work.tile([W, L * YPC], f32, tag="t2s")
                    nc.scalar.copy(t2s[:], t2x[:])

                    if not last:
                        qsb_nxt = work.tile([128, CH * CW], fp8, tag="qsb", bufs=2)
                        t1_nxt = work.tile([H, L * W], bf16, tag="t1full", bufs=2)
                        if it < 2:
                            # zero the pad columns once per buffer (lhsT
                            # reads 64-wide; NaN bit patterns must not leak)
                            qpad = (qsb_nxt[:]
                                    .rearrange("p (c b l) -> p c b l", b=2,
                                               l=32))
                            nc.vector.memset(qpad[:, :, 0, LE:32], 0.0)
                            nc.vector.memset(qpad[:, :, 1, LE:32], 0.0)

                    qy = work.tile([W, YPC * LE], f32, tag="qy")
                    qyv = qy[:].rearrange("x (y l) -> x y l", l=LE)
                    rec = work.tile([W, YPC], f32, tag="rec")
                    ssum = work.tile([W, YPC], f32, tag="ssum")
                    if not last:
                        qyb = work.tile([W, YPC * L], bf16, tag="qyb")
                        q8r8 = work.tile([W, YPC * NCH], fp8, tag="q8r8")
                        q8a = q8r8[:].rearrange("x (y h l) -> x y h l", h=2,
                                                l=LE)
                        q8v = q8a[:, :, 0, :]
                        r8v = q8a[:, :, 1, :]
                        t1s = work.tile([L, N // NCORES], bf16, tag="t1s")
                        if it == 0:
                            nc.vector.memset(q8v[:, :, L:LE], ONESV)
                            nc.vector.memset(r8v[:, :, L:LE], 0.0)

                    # fully batched tail: recombine all groups, 12
                    # transposes, then one wide op per stage
                    tp = ps_tp.tile([W, YPC * LE], f32, tag="tp")
                    tpv = tp[:].rearrange("x (y l) -> x y l", l=LE)
                    for g in range(GRP):
                        pevA = work.tile([LE, GS], f32, tag="pevA", bufs=2,
                                         name="pevA")
                        nc.scalar.copy(pevA[:], P0[g][0:LE, 0:GS])
                        pevB = work.tile([LE, GS], f32, tag="pevB", bufs=2,
                                         name="pevB")
                        nc.scalar.copy(pevB[:], P0[g][32:32 + LE, 0:GS])
                        pbs = work.tile([LE, GS], f32, tag="pbs", bufs=2,
                                        name="pbs")
                        nc.vector.tensor_tensor(pbs[:], pevA[:], pevB[:],
                                                OP.add)
                        if dbg and it == 0 and g == 0:
                            nc.sync.dma_start(dbg_pev.ap()[0:LE, :], pevA[:])
                            nc.sync.dma_start(dbg_pev.ap()[LE:2 * LE, :],
                                              pevB[:])
                        for k in range(GY):
                            y = g * GY + k
                            nc.tensor.transpose(tp[:, y * LE:(y + 1) * LE],
                                                pbs[:, k * W:(k + 1) * W],
                                                idn[:])
                    nc.vector.reciprocal(rec[:, :, None], tpv[:, :, L:LE])
                    nc.vector.tensor_tensor(
                        tpv[:], tpv[:],
                        rec[:, :, None].to_broadcast([W, YPC, LE]), OP.mult)
                    nc.vector.tensor_tensor(
                        tpv[:], tpv[:],
                        uTx[:].rearrange("x (y l) -> x y l", l=LE), OP.add)
                    nc.vector.tensor_tensor(
                        tpv[:, :, 0:L], tpv[:, :, 0:L],
                        t2s[:].rearrange("x (l y) -> x y l", y=YPC), OP.add)
                    nc.scalar.activation(qy[:], tp[:], AF.Exp)
                    nc.vector.reduce_sum(ssum[:], qyv[:, :, 0:L], axis=AX.X)
                    nc.vector.reciprocal(ssum[:], ssum[:])
                    nc.vector.tensor_tensor(
                        qyv[:, :, 0:L], qyv[:, :, 0:L],
                        ssum[:, :, None].to_broadcast([W, YPC, L]), OP.mult)
                    if dbg and it == 0:
                        nc.sync.dma_start(dbg_qy.ap(), qy[:])
                        nc.sync.dma_start(dbg_t2s.ap(), t2s[:])
                    if dbg and it == 0 and not last:
                        nc.vector.engine_nop()

                    if not last:
                        # conversions for x-blur + fp8 payload
                        nc.vector.tensor_copy(
                            qyb[:].rearrange("x (y l) -> x y l", l=L),
                            qyv[:, :, 0:L])
                        nc.vector.tensor_scalar(
                            q8v[:, :, 0:L], qyv[:, :, 0:L], QSC, None,
                            OP.mult)
                        nc.vector.scalar_tensor_tensor(
                            r8v[:, :, 0:L], qyv[:, :, 0:L], QSC,
                            q8v[:, :, 0:L], OP.mult, OP.subtract)
                        # x-blur -> t1 staging (bf16), one psum+copy per group
                        for g in range(GRP):
                            xb = ps_xb.tile([L, GS], f32, tag="xb")
                            for k in range(GY):
                                y = g * GY + k
                                nc.tensor.matmul(
                                    xb[:, k * W:(k + 1) * W],
                                    qyb[:, y * L:(y + 1) * L], Ax[:],
                                    start=True, stop=True)
                            nc.scalar.copy(
                                t1s[:, g * GS:(g + 1) * GS], xb[:])

                    if last:
                        nc.sync.dma_start(
                            qout_d.ap()
                            .rearrange("(y x) l -> x y l", x=W),
                            qyv[:, :, 0:L])
                        continue

                    # payload: q shuffle straight to DRAM (6 partition-shift
                    # pieces, all 3 groups folded) + one t1 DMA.  q and t1
                    # travel as separate NATIVE-dtype tensors: packing bf16
                    # pairs into f32 via bitcast corrupts values whose f32
                    # interpretation is special (seen as scattered inf/NaN in
                    # the low half of f32 lanes after the collective).
                    # single packed payload: q fp8 + t1 bf16 in one f32
                    # DRAM tensor (one collective, 8 modeled copies).  The
                    # t1 DMA keeps its PLAIN SBUF source AP (rearranged
                    # source reads miss subtile deps against the per-group
                    # x-blur copies); transpose strides live on the DRAM side.
                    pl = dram.tile([1, PAYP], f32, tag="pl")
                    plqv = (pl[0:1, 0:PQ_F32].bitcast(fp8)
                            .rearrange("a (p c l) -> (a p) c l", p=128,
                                       l=NCH))
                    for si, (yo, x0, n, co, p0) in enumerate(SHUF):
                        eng = nc.scalar if si % 2 else nc.sync
                        eng.dma_start(
                            plqv[p0:p0 + n, co::3, :],
                            q8r8[x0:x0 + n, :]
                            .rearrange("x (y hl) -> x y hl", hl=NCH)
                            [:, yo::GY, :])
                    nc.scalar.dma_start(
                        pl[0:1, PQ_F32:PAYP].bitcast(bf16)
                        .rearrange("a (y l x) -> l (a y) x", y=YPC, l=L),
                        t1s[:])

                    qag = dram.tile([NCORES, PAYP], f32, tag="qag")
                    if sim1:
                        for r in range(NCORES):
                            nc.sync.dma_start(qag[r:r + 1, :], pl[:])
                    else:
                        nc.gpsimd.collective_compute(
                            "AllGather", OP.bypass,
                            replica_groups=[list(range(NCORES))],
                            ins=[pl.opt()], outs=[qag.opt()])

                    # reloads for the next iteration, one q DMA per core (so
                    # the next burst starts as soon as core 0's slab lands)
                    for r in range(NCORES):
                        eng = nc.sync if r % 2 else nc.scalar
                        eng.dma_start(
                            qsb_nxt[:, 9 * r * CW:(9 * r + 9) * CW]
                            .rearrange("p (c b l) -> p c b l", b=2, l=32)
                            [:, :, :, 0:LE],
                            qag[r:r + 1, 0:PQ_F32].bitcast(fp8)
                            .rearrange("a (p c h l) -> (a p) c h l", p=128,
                                       h=2, l=LE))
                    nc.sync.dma_start(
                        t1_nxt[:],
                        qag[0:NCORES, PQ_F32:PAYP].bitcast(bf16)
                        .rearrange("r (y lx) -> r y lx", y=YPC))
                    if dbg and it == 0:
                        nc.vector.engine_nop()
                        nc.sync.dma_start(dbg_qsb.ap(), qsb_nxt[:])
                        nc.sync.dma_start(dbg_t1n.ap(), t1_nxt[:])
                        nc.sync.dma_start(dbg_t1s.ap(), t1s[:])
                        nc.sync.dma_start(dbg_qyb.ap(), qyb[:])
                    qsb_cur = qsb_nxt
                    t1_cur = t1_nxt
            psP_ctx.__exit__(None, None, None)

    nc.compile()
    _CACHE[key] = nc
    return nc


def _host_prepare(unaries, rgb):
    e4m3 = ml_dtypes.float8_e4m3
    bfd = ml_dtypes.bfloat16
    u = np.asarray(unaries, np.float32).reshape(N, L)
    c = np.asarray(rgb, np.float32).reshape(N, 3)

    ys, xs = np.meshgrid(np.arange(H, dtype=np.float64),
                         np.arange(W, dtype=np.float64), indexing="ij")
    pos = np.stack([ys.ravel(), xs.ravel()], -1)
    g = np.concatenate([c.astype(np.float64) / BETA, pos / ALPHA], 1)
    g = g - g.mean(0, keepdims=True)
    sq = (g * g).sum(1)
    ones = np.ones(N, np.float64)
    # y_ji = (SCB*g_j).g_i - SCB/2*sq_j + (-SCB/2*sq_i + C2)
    L7 = np.concatenate([(SCB * g).T, (-(SCB / 2) * sq)[None], ones[None]], 0)
    R7 = np.concatenate([g.T, ones[None], ((-(SCB / 2) * sq) + C2)[None]], 0)

    def split(A7):
        hi = A7.astype(bfd)
        lo = (A7 - hi.astype(np.float64)).astype(bfd)
        return hi, lo

    Lhi, Llo = split(L7)
    Rhi, Rlo = split(R7)
    featL = np.ascontiguousarray(np.concatenate([Lhi, Lhi, Llo], 0))  # [21,N]
    featR = np.ascontiguousarray(np.concatenate([Rhi, Rlo, Rhi], 0))

    d = np.arange(W, dtype=np.float64)
    A = np.exp(-(d[:, None] - d[None, :]) ** 2 / (2.0 * GAMMA * GAMMA))
    nvec = A.sum(0)
    Ax = np.ascontiguousarray((A / nvec[None, :]).astype(bfd))

    um = u.max(1, keepdims=True)
    e = np.exp(u - um)
    q0 = e / e.sum(1, keepdims=True)

    q8 = (QSC * q0).astype(e4m3)
    r8 = (QSC * q0 - q8.astype(np.float32)).astype(e4m3)
    q44 = np.zeros((N, CW), e4m3)
    q44[:, 0:L] = q8
    q44[:, L] = np.float32(ONESV)
    q44[:, 32:32 + L] = r8
    qsb0 = np.ascontiguousarray(
        q44.reshape(CH, 128, CW).transpose(1, 0, 2).reshape(128, CH * CW))

    q3 = q0.astype(bfd).astype(np.float64).reshape(H, W, L)
    t1 = np.einsum("Xx,yXl->ylx", A / nvec[None, :], q3)      # [96, 21, 96]
    t1f0 = np.ascontiguousarray(t1.reshape(H, L * W).astype(bfd))

    ident = np.eye(LE, dtype=np.float32)

    in_maps = []
    for core in range(NCORES):
        rows = slice(core * S, (core + 1) * S)
        yc = slice(core * YPC, (core + 1) * YPC)
        Ay_c = np.ascontiguousarray(
            (A[:, yc] * (W_SPATIAL / nvec[yc])[None, :]).astype(bfd))
        # x-major unaries: uTx[x, y_loc*22 + l], channel 21 = -50
        uc = u[rows].reshape(YPC, W, L)
        uTx = np.full((W, YPC, LE), -50.0, np.float32)
        uTx[:, :, 0:L] = uc.transpose(1, 0, 2)
        in_maps.append({
            "featL": featL,
            "featR": np.ascontiguousarray(featR[:, rows]),
            "uTx": np.ascontiguousarray(uTx.reshape(W, YPC * LE)),
            "Ax": Ax,
            "Ay": Ay_c,
            "qsb0": qsb0,
            "t1f0": t1f0,
            "ident": ident,
        })
    return in_maps


def _get_runner():
    if "runner" in _CACHE:
        return _CACHE["runner"]
    import jax
    from jax.sharding import Mesh, PartitionSpec
    from jax.experimental.shard_map import shard_map
    import concourse.mybir as mybir
    from concourse import bass2jax

    nc = _build_bass()
    if not DBG_SIM:
        bass2jax.install_neuronx_cc_hook()

    partition_name = (nc.partition_id_tensor.name
                      if nc.partition_id_tensor else None)
    in_names, out_names, out_avals, zero_outs = [], [], [], []
    for alloc in nc.m.functions[0].allocations:
        if not isinstance(alloc, mybir.MemoryLocationSet):
            continue
        name = alloc.memorylocations[0].name
        if alloc.kind == "ExternalInput":
            if name != partition_name:
                in_names.append(name)
        elif alloc.kind == "ExternalOutput":
            shape = tuple(alloc.tensor_shape)
            dtype = mybir.dt.np(alloc.dtype)
            out_names.append(name)
            out_avals.append(jax.core.ShapedArray(shape, dtype))
            zero_outs.append(np.zeros(shape, dtype))
    n_params = len(in_names)
    all_in_names = list(in_names) + list(out_names)
    if partition_name is not None:
        all_in_names.append(partition_name)

    def _body(*args):
        operands = list(args)
        if partition_name is not None:
            operands.append(bass2jax.partition_id_tensor())
        outs = bass2jax._bass_exec_p.bind(
            *operands,
            out_avals=tuple(out_avals),
            in_names=tuple(all_in_names),
            out_names=tuple(out_names),
            lowering_input_output_aliases=(),
            sim_require_finite=DBG_SIM,
            sim_require_nnan=False,
            nc=nc,
        )
        return tuple(outs)

    devices = jax.devices()[:NCORES]
    mesh = Mesh(np.asarray(devices), ("core",))
    n_outs = len(out_names)
    in_specs = (PartitionSpec("core"),) * (n_params + n_outs)
    out_specs = (PartitionSpec("core"),) * n_outs
    donate = tuple(range(n_params, n_params + n_outs))
    fn = jax.jit(
        shard_map(_body, mesh=mesh, in_specs=in_specs, out_specs=out_specs,
                  check_rep=False),
        donate_argnums=donate, keep_unused=True)
    _CACHE["runner"] = (fn, in_names, out_names, out_avals, zero_outs)
    return _CACHE["runner"]


def _concat_inputs(in_maps, in_names):
    return [np.concatenate([np.asarray(in_maps[c][nm]) for c in range(NCORES)],
                           axis=0) for nm in in_names]


def _run(in_maps):
    fn, in_names, out_names, out_avals, zero_outs = _get_runner()
    concat_in = _concat_inputs(in_maps, in_names)
    concat_zeros = [np.zeros((NCORES * z.shape[0], *z.shape[1:]), z.dtype)
                    for z in zero_outs]
    out_arrs = fn(*concat_in, *concat_zeros)
    return out_arrs, out_names, out_avals


def kernel(unaries, rgb):
    in_maps = _host_prepare(unaries, rgb)
    out_arrs, out_names, out_avals = _run(in_maps)
    qi = out_names.index("qout")
    q = np.asarray(out_arrs[qi]).reshape(NCORES, S, L).reshape(N, L)
    return np.ascontiguousarray(q[None].astype(np.float32))


def time_kernel(unaries, rgb, iters=20):
    """Steady-state per-call wall time of the compiled 8-core executable."""
    import time as _time
    import jax
    in_maps = _host_prepare(unaries, rgb)
    fn, in_names, out_names, out_avals, zero_outs = _get_runner()
    concat_in = _concat_inputs(in_maps, in_names)

    def once():
        concat_zeros = [np.zeros((NCORES * z.shape[0], *z.shape[1:]), z.dtype)
                        for z in zero_outs]
        outs = fn(*concat_in, *concat_zeros)
        jax.block_until_ready(outs)
        return outs

    once()
    times = []
    for _ in range(iters):
        t0 = _time.perf_counter()
        once()
        times.append(_time.perf_counter() - t0)
    return min(times), sorted(times)[len(times) // 2]
